# revision 7
# baseline (speedup 1.0000x reference)
"""Llama GQA attention layer (T=2048, H=4096, 32 q heads / 8 kv heads, hd=128),
tensor-parallel over heads across 8 Trainium2 NeuronCores.

Per core c: 4 q heads + 1 kv head (wq/wk/wv column slices, wo row slice).
Each core computes a full [T, H] partial o_proj output; partials are summed on
host (the all-reduce of the TP scheme).

Design (vs the 534us fp32r baseline; measures ~447-455us on a warm chip):
- Everything in HBM and SBUF is bf16 (same 1 cyc/row PE rate as fp32r, half
  the DMA bytes; fp32 PSUM accumulation; rel-err ~3.6e-3 vs 2e-2 budget).
- DMA-packet-friendly layouts: k-major ht [32,128,2048] (4KB rows), packed
  wide weight tiles, 4-chunk-wide output staging; startup streams the
  first-needed pieces first; wo prefetched during phase 1.
- ht is streamed in per-t-chunk column halves with lookahead, then re-filled
  in-place (pass 1) for t-chunks {2,3}, so SBUF holds only 8MB of ht.
- Per-t-chunk qT/kT/v tiles and per-head attn_q tiles keep the Tile
  dependency tracker from serializing phase boundaries on whole-tile deps.
- Causal diagonal slicing: the 4 diagonal k-chunks of each q-chunk compute
  exp/PV/den only for columns q >= kc*128 (~15% less attention work); the
  in-block triangle is masked with one [128,128] multiply.
- Softmax denominator: batched ones-matmuls (one per chunk, back-to-back per
  head) into a [1,TC] PSUM bank; reciprocal on DVE; partition-broadcast on
  the otherwise-idle GpSimd engine (library pre-warmed in phase 1); the
  normalize multiply is fused into the attention PSUM drain and deferred one
  head so no engine queue head-blocks.
- o_proj interleaved one q-chunk behind attention; drains on DVE while
  attention runs (Act stays exp-only); the final q-chunk rotates its PSUM
  accumulators over 6 banks and splits drains Act/DVE with per-chunk output
  DMAs to minimize the kernel tail.
"""

import sys

if "/opt/trn_rl_repo" not in sys.path:
    sys.path.insert(0, "/opt/trn_rl_repo")

import numpy as np

import concourse.bass as bass
import concourse.bacc as bacc
import concourse.tile as tile
import concourse.mybir as mybir
from concourse import bass_utils

T = 2048
H = 4096
NQ = 32
NKV = 8
HD = 128
THETA = 10000.0
N_CORES = 8
NH = NQ // N_CORES          # local q heads per core
HALF = HD // 2
TC = 512                    # t-chunk (matmul free dim)
NTC = T // TC               # 4
NKCH = H // 128             # 32 hidden chunks
NMO = H // 128              # 32 o_proj output chunks
OG = 4                      # o_proj mo chunks per output staging group
TRAIL = 3                   # PV lag behind the score stream (chunks)
SCALE = float(HD) ** -0.5

F32 = mybir.dt.float32
BF16 = mybir.dt.bfloat16
ALU = mybir.AluOpType
ACTF = mybir.ActivationFunctionType


def _build():
    nc = bacc.Bacc("TRN2", target_bir_lowering=False, debug=False,
                   num_devices=N_CORES)
    ht = nc.dram_tensor("ht", [NKCH, 128, T], BF16, kind="ExternalInput").ap()
    wq = nc.dram_tensor("wq", [128, NKCH * NH * 128], BF16,
                        kind="ExternalInput").ap()
    wk = nc.dram_tensor("wk", [128, NKCH * 128], BF16, kind="ExternalInput").ap()
    wv = nc.dram_tensor("wv", [128, NKCH * 128], BF16, kind="ExternalInput").ap()
    wo = nc.dram_tensor("wo", [NH, 128, H], BF16, kind="ExternalInput").ap()
    cos2 = nc.dram_tensor("cos2", [128, T], BF16, kind="ExternalInput").ap()
    sin2 = nc.dram_tensor("sin2", [128, T], BF16, kind="ExternalInput").ap()
    ident = nc.dram_tensor("ident", [128, 128], BF16, kind="ExternalInput").ap()
    ones = nc.dram_tensor("ones", [128, 1], BF16, kind="ExternalInput").ap()
    tri = nc.dram_tensor("tri", [128, 128], BF16, kind="ExternalInput").ap()
    out_t = nc.dram_tensor("out_t", [NTC * (NMO // OG), 128, OG * TC], BF16,
                           kind="ExternalOutput").ap()

    with tile.TileContext(nc) as tc:
        _body(tc, ht, wq, wk, wv, wo, cos2, sin2, ident, ones, tri, out_t)
    nc.compile()
    return nc


def _oproj(nc, op_pools, ost, wo_sb, attn_q, out_t, qc, last=False):
    """out[mo, t] = sum_h wo[f, mo] * attnT[f, t] for t-chunk qc; outputs are
    staged 4 mo-chunks wide so each DRAM write has 4KB rows.  For the last
    q-chunk the drains are split Act/DVE and the DMAs go per half-group so
    the kernel tail is as short as possible."""
    for g in range(NMO // OG):
        stg = ost.tile([128, OG * TC], BF16, tag="ostg", name="ostg")
        for i in range(OG):
            mo = g * OG + i
            pool, ptag = op_pools[mo % len(op_pools)]
            op = pool.tile([128, TC], F32, tag=ptag, name=ptag)
            for h in range(NH):
                nc.tensor.matmul(
                    op[:],
                    wo_sb[h][:, mo * 128:(mo + 1) * 128],
                    attn_q[qc][h][:],
                    start=(h == 0), stop=(h == NH - 1))
            dst = stg[:, i * TC:(i + 1) * TC]
            if last:
                # split the drain so the PSUM bank frees in half the time
                nc.scalar.copy(dst[:, 0:TC // 2], op[:, 0:TC // 2])
                nc.vector.tensor_copy(dst[:, TC // 2:TC], op[:, TC // 2:TC])
                if g == NMO // OG - 1:
                    nc.sync.dma_start(
                        out_t[qc * (NMO // OG) + g][:, i * TC:(i + 1) * TC],
                        stg[:, i * TC:(i + 1) * TC])
                elif i % 2 == 1:
                    half = slice((i - 1) * TC, (i + 1) * TC)
                    nc.sync.dma_start(
                        out_t[qc * (NMO // OG) + g][:, half], stg[:, half])
            else:
                nc.vector.tensor_copy(dst, op[:])
        if not last:
            nc.sync.dma_start(out_t[qc * (NMO // OG) + g], stg[:])


def _body(tc, ht, wq, wk, wv, wo, cos2, sin2, ident, ones, tri, out_t):
    nc = tc.nc

    with (
        tc.tile_pool(name="persist", bufs=1) as persist,
        tc.tile_pool(name="small", bufs=1) as small,
    ):
        # per-t-chunk tiles so readers only depend on the producing t-chunk
        qT_t = [persist.tile([128, NH * TC], BF16, tag=f"qT{t}", name=f"qT{t}")
                for t in range(NTC)]     # [d, h*TC + tc]
        kT_t = [persist.tile([128, TC], BF16, tag=f"kT{t}", name=f"kT{t}")
                for t in range(NTC)]     # [d, tc]
        v_t = [persist.tile([128, TC], BF16, tag=f"v{t}", name=f"v{t}")
               for t in range(NTC)]      # chunk i cols: v[t=i*128+p, d]
        wo_sb = [persist.tile([128, H], BF16, tag=f"wo{h}", name=f"wo{h}")
                 for h in range(NH)]
        ident_sb = small.tile([128, 128], BF16, tag="ident")
        ones_sb = small.tile([128, 1], BF16, tag="ones")
        tri_sb = small.tile([128, 128], BF16, tag="tri")        # tri[k,q]=1 if q>=k
        warm_sb = small.tile([128, 128], BF16, tag="warm")

        # ---------------- phase 1: QKV projections + RoPE + V transpose ----
        with (
            tc.tile_pool(name="ph1w", bufs=1) as ph1w,
            tc.tile_pool(name="ph1h", bufs=1) as ph1h,
            tc.tile_pool(name="rope", bufs=2) as rope,
            tc.tile_pool(name="ps1", bufs=1, space="PSUM") as ps1,
            tc.tile_pool(name="pst", bufs=2, space="PSUM") as pst,
        ):
            cos_sb = ph1w.tile([128, T], BF16, tag="cos")
            sin_sb = ph1w.tile([128, T], BF16, tag="sin")
            vT_t = [ph1w.tile([128, TC], BF16, tag=f"vT{t}", name=f"vT{t}")
                    for t in range(NTC)]
            wq_sb = ph1w.tile([128, NKCH * NH * 128], BF16, tag="wq")
            wk_sb = ph1w.tile([128, NKCH * 128], BF16, tag="wk")
            wv_sb = ph1w.tile([128, NKCH * 128], BF16, tag="wv")
            # one SBUF tile per hidden chunk, holding a t-pair (2*TC cols);
            # pass 0 covers t-chunks {0,1}, the re-DMA (pass 1) covers {2,3}
            ht_sb = [ph1h.tile([128, 2 * TC], BF16, tag=f"ht{k}", name=f"ht{k}")
                     for k in range(NKCH)]

            QW = NH * 128  # wq cols per hidden chunk
            for t in range(NTC):
                toff = (t % 2) * TC
                qps = [ps1.tile([128, TC], F32, tag=f"qps{fc}", name=f"qps{fc}")
                       for fc in range(NH)]
                kps = ps1.tile([128, TC], F32, tag="kps")
                vps = ps1.tile([128, TC], F32, tag="vps")
                for k in range(NKCH):
                    if t == 0:
                        # t0 only streams the first halves (its own columns)
                        # with a 4-chunk lookahead; the t1 halves follow from
                        # k>=10 so t0's DMA demand stays under the line rate
                        if k == 0:
                            # first MM needs exactly ht[0] halfA + wq chunk 0:
                            # issue those two first
                            nc.sync.dma_start(ht_sb[0][:, 0:TC], ht[0][:, 0:TC])
                            nc.sync.dma_start(wq_sb[:, 0:QW], wq[:, 0:QW])
                            nc.sync.dma_start(wk_sb[:, 0:1024], wk[:, 0:1024])
                            nc.sync.dma_start(wv_sb[:, 0:1024], wv[:, 0:1024])
                            for j in range(1, 4):
                                nc.sync.dma_start(ht_sb[j][:, 0:TC],
                                                  ht[j][:, 0:TC])
                            nc.sync.dma_start(wq_sb[:, QW:4 * QW],
                                              wq[:, QW:4 * QW])
                        if k + 4 < NKCH:
                            nc.sync.dma_start(ht_sb[k + 4][:, 0:TC],
                                              ht[k + 4][:, 0:TC])
                        if k >= 10:
                            nc.sync.dma_start(ht_sb[k - 10][:, TC:2 * TC],
                                              ht[k - 10][:, TC:2 * TC])
                        if k == 1:
                            nc.sync.dma_start(ident_sb[:], ident[:, :])
                        if k == 2:
                            nc.sync.dma_start(wq_sb[:, 4 * QW:16 * QW],
                                              wq[:, 4 * QW:16 * QW])
                        if k == 6:
                            nc.sync.dma_start(wk_sb[:, 1024:4096],
                                              wk[:, 1024:4096])
                            nc.sync.dma_start(wv_sb[:, 1024:4096],
                                              wv[:, 1024:4096])
                        if k == 8:
                            nc.sync.dma_start(cos_sb[:], cos2[:, :])
                            nc.sync.dma_start(sin_sb[:], sin2[:, :])
                        if k == 10:
                            nc.sync.dma_start(wq_sb[:, 16 * QW:32 * QW],
                                              wq[:, 16 * QW:32 * QW])
                        if k == 12:
                            nc.sync.dma_start(ones_sb[:], ones[:, :])
                            nc.sync.dma_start(tri_sb[:], tri[:, :])
                        if k == 31:
                            # load the GpSimd library + Act exp table ahead
                            # of first use at the phase boundary
                            nc.gpsimd.partition_broadcast(
                                warm_sb[:], tri_sb[0:1, :], channels=128)
                            nc.scalar.activation(warm_sb[0:1, 0:16],
                                                 cos_sb[0:1, 0:16], ACTF.Exp,
                                                 scale=SCALE)
                    if t == 1 and k < 10:
                        nc.sync.dma_start(ht_sb[k + 22][:, TC:2 * TC],
                                          ht[k + 22][:, TC:2 * TC])
                    if t == 3 and k % 4 == 0:
                        # prefetch wo during phase 1 so o_proj never waits
                        j = k // 4
                        hh, cc = j // 2, (j % 2) * (H // 2)
                        nc.sync.dma_start(wo_sb[hh][:, cc:cc + H // 2],
                                          wo[hh][:, cc:cc + H // 2])
                    st, sp = (k == 0), (k == NKCH - 1)
                    mv = ht_sb[k][:, toff:toff + TC]
                    for fc in range(NH):
                        nc.tensor.matmul(
                            qps[fc][:],
                            wq_sb[:, k * QW + fc * 128:k * QW + (fc + 1) * 128],
                            mv, start=st, stop=sp)
                    nc.tensor.matmul(kps[:], wk_sb[:, k * 128:(k + 1) * 128],
                                     mv, start=st, stop=sp)
                    nc.tensor.matmul(vps[:], wv_sb[:, k * 128:(k + 1) * 128],
                                     mv, start=st, stop=sp)
                    if t == 1:
                        # pass-1 prefetch: re-fill this chunk with t-chunks {2,3}
                        nc.sync.dma_start(ht_sb[k][:], ht[k][:, 2 * TC:4 * TC])

                # RoPE: out = x*cos2 + swap(x)*sin2   (swap = halves exchanged)
                for hc in range(NH + 1):
                    src_ps = qps[hc] if hc < NH else kps
                    dst = (qT_t[t][:, hc * TC:(hc + 1) * TC]
                           if hc < NH else kT_t[t][:])
                    raw = rope.tile([128, TC], BF16, tag="raw")
                    # at the phase boundary, split the PSUM drains between
                    # Act and DVE so the first exps aren't queued behind them
                    if t == NTC - 1 and hc % 2 == 1:
                        nc.vector.tensor_copy(raw[:], src_ps[:])
                    else:
                        nc.scalar.copy(raw[:], src_ps[:])
                    sw = rope.tile([128, TC], BF16, tag="sw")
                    nc.sync.dma_start(sw[0:HALF, :], raw[HALF:128, :])
                    nc.sync.dma_start(sw[HALF:128, :], raw[0:HALF, :])
                    a = rope.tile([128, TC], BF16, tag="ra")
                    b = rope.tile([128, TC], BF16, tag="rb")
                    nc.vector.tensor_tensor(
                        a[:], raw[:], cos_sb[:, t * TC:(t + 1) * TC], ALU.mult)
                    nc.vector.tensor_tensor(
                        b[:], sw[:], sin_sb[:, t * TC:(t + 1) * TC], ALU.mult)
                    nc.vector.tensor_tensor(dst, a[:], b[:], ALU.add)
                # V has no rope; stash vT then transpose to natural layout
                nc.scalar.copy(vT_t[t][:], vps[:])
                for i in range(4):
                    tp = pst.tile([128, 128], BF16, tag="tp")
                    nc.tensor.transpose(tp[:], vT_t[t][:, i * 128:(i + 1) * 128],
                                        ident_sb[:])
                    if i % 2 == 0:
                        nc.vector.tensor_copy(
                            v_t[t][:, i * 128:(i + 1) * 128], tp[:])
                    else:
                        nc.scalar.copy(
                            v_t[t][:, i * 128:(i + 1) * 128], tp[:])

        # ------- phase 2+3: attention interleaved with o_proj, per q-chunk --
        with (
            tc.tile_pool(name="att", bufs=17) as att,
            tc.tile_pool(name="late", bufs=1) as late,
            tc.tile_pool(name="rbp", bufs=2) as rbp,
            tc.tile_pool(name="ost", bufs=2) as ost,
            tc.tile_pool(name="ps_s", bufs=2, space="PSUM") as ps_s,
            tc.tile_pool(name="ps_o", bufs=2, space="PSUM") as ps_o,
            tc.tile_pool(name="ps_d", bufs=2, space="PSUM") as ps_d,
            tc.tile_pool(name="ps_op", bufs=2, space="PSUM") as ps_op,
        ):
            attn_q = [[late.tile([128, TC], BF16, tag=f"attnq{i}_{h}",
                                 name=f"attnq{i}_{h}") for h in range(NH)]
                      for i in range(NTC)]

            # deferred normalize+drain: one head behind the attention stream
            pending = []

            def _flush_norm():
                qcp, hp, po_p, rb_p = pending.pop(0)
                nc.vector.tensor_tensor(
                    attn_q[qcp][hp][:], po_p[:], rb_p[:], ALU.mult)

            for qc in range(NTC):
                nkc = (qc + 1) * (TC // 128)     # causal k chunks of 128
                for h in range(NH):
                    qh = qT_t[qc][:, h * TC:(h + 1) * TC]
                    po = ps_o.tile([128, TC], F32, tag="po")
                    pd = ps_d.tile([1, TC], F32, tag="pd")
                    ps = []
                    offs = []

                    def _pv(kc):
                        off = offs[kc]
                        st, sp = (kc == 0), (kc == nkc - 1)
                        vsl = v_t[kc // 4][:, (kc % 4) * 128:(kc % 4 + 1) * 128]
                        nc.tensor.matmul(po[:, off:TC], vsl,
                                         ps[kc][:, off:TC], start=st, stop=sp,
                                         skip_group_check=True)

                    for kc in range(nkc):
                        di = kc - (nkc - 4)
                        # diagonal chunks only need columns q >= kc*128
                        off = di * 128 if di > 0 else 0
                        offs.append(off)
                        ksl = kT_t[kc // 4][:, (kc % 4) * 128:(kc % 4 + 1) * 128]
                        sT = ps_s.tile([128, TC], F32, tag="sT")
                        nc.tensor.matmul(sT[:, off:TC], ksl,
                                         qh[:, off:TC], start=True, stop=True)
                        p = att.tile([128, TC], BF16, tag="p", name="p")
                        nc.scalar.activation(p[:, off:TC], sT[:, off:TC],
                                             ACTF.Exp, scale=SCALE)
                        if di >= 0:
                            # zero the upper triangle of the diagonal block
                            dsl = slice(di * 128, (di + 1) * 128)
                            nc.vector.tensor_tensor(
                                p[:, dsl], p[:, dsl], tri_sb[:], ALU.mult)
                        ps.append(p)
                        # PV trails the score stream so the PE never waits on
                        # the exp chain
                        if kc >= TRAIL:
                            _pv(kc - TRAIL)
                    for kc in range(max(0, nkc - TRAIL), nkc):
                        _pv(kc)
                    # denominator: one ones-LDWEIGHTS, then back-to-back
                    # accumulation over all chunks of this head
                    for kc in range(nkc):
                        off = offs[kc]
                        nc.tensor.matmul(pd[:, off:TC], ones_sb[:],
                                         ps[kc][:, off:TC], start=(kc == 0),
                                         stop=(kc == nkc - 1),
                                         skip_group_check=True)
                    # reciprocal of the denominator (DVE) + broadcast across
                    # partitions on the idle GpSimd engine; the normalize
                    # multiply is deferred one head so the DVE never blocks
                    rc = rbp.tile([1, TC], F32, tag="rc")
                    nc.vector.reciprocal_approx_fast(out=rc[:], in_=pd[:])
                    rb = rbp.tile([128, TC], F32, tag="rb")
                    nc.gpsimd.partition_broadcast(rb[:], rc[:], channels=128)
                    if pending:
                        _flush_norm()
                    pending.append((qc, h, po, rb))
                if qc >= 1:
                    _oproj(nc, [(ps_op, "op")], ost, wo_sb, attn_q, out_t,
                           qc - 1)
            _flush_norm()
            # attention pools are idle now: rotate op tiles over 6 banks so
            # the drain never gates the matmul stream
            _oproj(nc, [(ps_op, "op"), (ps_o, "po"), (ps_s, "sT")], ost,
                   wo_sb, attn_q, out_t, NTC - 1, last=True)


_NC = None
LAST_EXEC_NS = None
LAST_TRACE = None


def _ensure_profile_hook():
    """Register the axon NTFF profiling hook (container lacks antenv.axon_hooks)."""
    import types
    import antenv
    if "antenv.axon_hooks" in sys.modules:
        return
    hooks_mod = types.ModuleType("antenv.axon_hooks")
    _h = [None]
    hooks_mod.set_axon_ntff_profile_hook = lambda hk: _h.__setitem__(0, hk)
    hooks_mod.get_axon_ntff_profile_hook = lambda: _h[0]
    sys.modules["antenv.axon_hooks"] = hooks_mod
    antenv.axon_hooks = hooks_mod
    from trn_agent_boot.trn_boot import _ntff_profile_via_ctypes
    hooks_mod.set_axon_ntff_profile_hook(
        _ntff_profile_via_ctypes("/opt/axon/libaxon_pjrt.so"))
    bass_utils.upload_artifacts = lambda tmpdir: "local://skipped"


def kernel(positions, hidden_states, wq, wk, wv, wo, _trace=False, **_unused):
    global _NC, LAST_EXEC_NS, LAST_TRACE
    import ml_dtypes
    bf16 = ml_dtypes.bfloat16
    positions = np.asarray(positions)
    hidden_states = np.asarray(hidden_states, dtype=np.float32)
    wq = np.asarray(wq, dtype=np.float32)
    wk = np.asarray(wk, dtype=np.float32)
    wv = np.asarray(wv, dtype=np.float32)
    wo = np.asarray(wo, dtype=np.float32)

    # host-side input prep (sharding + layout)
    hT = hidden_states.T                                            # [H, T]
    # k-major layout [k, 128, T]: every per-partition DMA row is 4KB (bf16)
    ht_b = np.ascontiguousarray(hT.reshape(NKCH, 128, T)).astype(bf16)
    inv_freq = (1.0 / (THETA ** (np.arange(HALF, dtype=np.float64) / HALF)))
    ang = positions.astype(np.float64)[:, None] * inv_freq[None, :]  # [T, 64]
    cos = np.cos(ang).astype(np.float32).T                           # [64, T]
    sin = np.sin(ang).astype(np.float32).T
    cos2 = np.ascontiguousarray(np.concatenate([cos, cos], axis=0)).astype(bf16)
    sin2 = np.ascontiguousarray(np.concatenate([-sin, sin], axis=0)).astype(bf16)
    ident = np.eye(128, dtype=np.float32).astype(bf16)
    dk = np.arange(128)[:, None]
    dq = np.arange(128)[None, :]
    tri = (dq >= dk).astype(np.float32).astype(bf16)   # [128,128] causal block

    def pack_w(w_slice):
        # [H, C] -> [128, NKCH*C]: row p holds chunk-k cols at k*C+j
        C = w_slice.shape[1]
        return np.ascontiguousarray(
            w_slice.reshape(NKCH, 128, C).transpose(1, 0, 2).reshape(
                128, NKCH * C)).astype(bf16)

    in_maps = []
    for c in range(N_CORES):
        in_maps.append({
            "ht": ht_b,
            "wq": pack_w(wq[:, c * NH * HD:(c + 1) * NH * HD]),
            "wk": pack_w(wk[:, c * HD:(c + 1) * HD]),
            "wv": pack_w(wv[:, c * HD:(c + 1) * HD]),
            "wo": np.ascontiguousarray(
                wo[c * NH * HD:(c + 1) * NH * HD, :].reshape(NH, 128, H)
            ).astype(bf16),
            "cos2": cos2,
            "sin2": sin2,
            "ident": ident,
            "ones": np.ones((128, 1), dtype=np.float32).astype(bf16),
            "tri": tri,
        })

    if _NC is None:
        _NC = _build()
    if _trace:
        _ensure_profile_hook()
    res = bass_utils.run_bass_kernel_spmd(
        _NC, in_maps, core_ids=list(range(N_CORES)), trace=_trace)
    if _trace:
        LAST_EXEC_NS = res.exec_time_ns
        LAST_TRACE = (res.instructions_and_trace[1]
                      if res.instructions_and_trace else None)

    acc = res.results[0]["out_t"].astype(np.float32)
    for c in range(1, N_CORES):
        acc += res.results[c]["out_t"].astype(np.float32)
    # [(qc*8+g), 128, 4*TC] -> out_ht[(g*4+i)*128+p, qc*TC+j]
    a = acc.reshape(NTC, NMO // OG, 128, OG, TC)
    out_ht = a.transpose(1, 3, 2, 0, 4).reshape(H, T)
    return np.ascontiguousarray(out_ht.T).astype(np.float32)
